# revision 49
# baseline (speedup 1.0000x reference)
"""Trainium2 Bass kernel for nn_DenseBlock_MHSA (dense_cnn).

Data-parallel over batch across 8 NeuronCores (512 samples/core).
Layout: channel-major activations [ch, tokens] on chip, tokens = 512*9 = 4608/core.
All GEMMs run fp16 (full PE rate), accumulation fp32 in PSUM. Per-sample 9x9
attention is batched 14 samples/group as masked 126x126 blocks; the softmax
mask bakes in a -9 logit shift so fp16 exp cannot overflow.

Optimizations over the 588-608us baseline (measured ~542us, rel err 1.4e-3;
fp8/DoubleRow was evaluated and rejected: e4m3's ~3.6%/operand quantization
noise gives 3.5-10% final error vs the 2e-2 gate, and exact hi+lo
compensation needs more DoubleRow passes than plain fp16 streaming):
- conv1 folded into the MHSA1 q/k/v projections: o3 = W1 a1 is consumed only
  by the first attention's channel GEMMs, so q = (Wq W1) a1 etc. with
  host-composited weights. Removes one full 512x512 GEMM unit (~38us of PE
  streaming) plus the o3 psum evacuations.
- bn1/bn2/bn3 carry identical gamma/beta on the x channels, so the
  pre-activated x is ONE tensor shared by all three phases: the conv2x and
  conv3x window convs share a single load per chunk, and chunk tiles still
  resident from the previous phase seed each window loop (the AR1 window
  starts DMA-free, halving its DMA pressure).
- AR seams: the all-reduce is kicked before the (queue-blocking) result
  readback and the raw q/k/v weight reloads (which reuse the dead composite
  tiles' SBUF); stats_post runs after the window work so its blocked vector
  ops cannot gate psum evacuations; small keep-warm PE fillers bridge the
  residual cross-core-skew dead zone so phase 2 doesn't restart at HAM 4/8.
- stats_post rsqrt on the vector engine (quake seed on uint32-bitcast views
  + 1 Newton step; uint add saturates on the DVE so the magic constant is
  applied via bitwise_not + two subtracts) instead of scalar Sqrt, whose
  ACT_TABLE_LOAD cost 4x 1.28us mid-kernel scalar stalls at the seams.
- startup: composite q/k weights k-sliced alternating sync/scalar queues,
  first (small) x chunk on gpsimd ahead of the consts, second full chunk on
  scalar; output DMAs rotate per-tile over three queues and phase 3 ends on
  the small remainder chunk so the final drain is ~1us.
- psum evacuations balanced across Scalar and Vector per phase; L psum
  double-buffered so head h+1 scores overlap head h softmax; relu'd
  attention inputs double-buffered.
- v-bias dropped: BN is shift-invariant, so the conv-v bias cancels exactly
  in every consumer of the attention outputs (same for the folded W1 bias
  through v).
"""

import numpy as np

import concourse.bass as bass
import concourse.mybir as mybir
import concourse.tile as tile
from concourse import bacc
from concourse.bass_utils import run_bass_kernel_spmd

F32 = mybir.dt.float32
FP16 = mybir.dt.float16
AF = mybir.ActivationFunctionType
ALU = mybir.AluOpType

N_CORES = 8
B, C, F, HW = 4096, 512, 512, 9
HEADS, D = 4, 128
BC = B // N_CORES            # samples per core
T = BC * HW                  # tokens per core
EPS = 1e-5
NTOT = float(B * HW)         # global token count for BN stats

CH_S = 56                    # samples per chunk
NT_F = CH_S * HW             # 504 tokens per full chunk
GS = 14 * HW                 # 126 tokens per attention group
CHUNKS = [(c * CH_S, CH_S) for c in range(BC // CH_S)]
if BC % CH_S:
    CHUNKS.append(((BC // CH_S) * CH_S, BC % CH_S))
NCH = len(CHUNKS)
EARLY_SET = (8, 7, 0, 1)     # chunks whose conv3 x-half runs in the AR1 window
WIN_ORDER = (8, 7, 0, 1, 2, 3, 4, 5, 9, 6)    # conv2x/conv3x window order
AR2WIN_ORDER = (9, 6, 8, 7, 0, 1, 2, 3, 4, 5)  # conv3 a-half (+x) order

VEC = {n: i for i, n in enumerate(
    ["qb", "kb", "b2", "b3",
     "g2a", "b2a", "g3a", "b3a", "g3b", "b3b", "c6"])}
NV = len(VEC)

_cache = {}


def _build():
    nc = bacc.Bacc("TRN2", target_bir_lowering=False, debug=False,
                   num_devices=N_CORES)

    dram = {}

    def din(name, shape, dt):
        dram[name] = nc.dram_tensor(name, shape, dt, kind="ExternalInput").ap()
        return dram[name]

    # bn1/bn2/bn3 share gamma=1, beta=0 on the x channels, so the
    # pre-activated x is one tensor shared by all three phases.
    xn_d = din("xn", [128, 4, T], FP16)
    w_d = {n: din(n, [128, 4 * F], FP16)
           for n in ["wqc", "wkc", "wvc", "wq", "wk", "wv",
                     "w2x", "w2a", "w3x", "w3a", "w3b"]}
    vecs_d = din("vecs", [128, 4 * NV], F32)
    vec4_d = din("vec4", [128, 24], F32)
    posrep_d = din("posrep", [D, HEADS * GS], FP16)
    blk_d = din("blkmask", [GS, NT_F], FP16)
    ones_d = din("ones126", [GS, 128], FP16)
    out_d = nc.dram_tensor("out_cm", [128, 4, T], FP16, kind="ExternalOutput").ap()

    with tile.TileContext(nc) as tc:
        from contextlib import ExitStack
        es = ExitStack()
        cpool = es.enter_context(tc.tile_pool(name="consts", bufs=1))
        apool = es.enter_context(tc.tile_pool(name="attres", bufs=1))
        dpool = es.enter_context(tc.tile_pool(name="dram", bufs=1, space="DRAM"))
        work = es.enter_context(tc.tile_pool(name="work", bufs=2))
        ps = es.enter_context(tc.tile_pool(name="ps", bufs=2, space="PSUM"))

        # ---- startup-critical loads.  sync: composite q weights (first
        # GEMM), scalar/vector queues: composite k/v weights, gpsimd: first
        # (small) x chunk then small consts.  Raw + phase-2/3 weights are
        # DMA'd later, once phase 1 is underway. ----
        def gload(name, dr, shape, dt, eng=None):
            t_ = cpool.tile(shape, dt, name=name, tag=name)
            (eng or nc.gpsimd).dma_start(t_[:], dr[:])
            return t_

        # PE pre-warm: the HAM holds the PE at 4/8 (1.2 GHz) until it sees
        # ~3.4us of sustained activity, so without this the first ~20us of
        # real matmuls run at half clock.  Stream zero matmuls from a
        # memset tile while the startup DMAs are still in flight.
        wrm = cpool.tile([128, 640], FP16, name="wrm", tag="wrm")
        nc.vector.memset(wrm[:], 0.0)
        for wg in range(10):
            fp_ = ps.tile([128, NT_F], F32, name="fillw", tag="mmps", bufs=4)
            nc.tensor.matmul(fp_[:, :NT_F], wrm[:, 0:128], wrm[:, 128:632],
                             start=True, stop=True)

        # first chunk of phase 1 is the small remainder chunk: its load goes
        # first on sync (gpsimd's first DMA pays ring warmup) so the first
        # real matmul is gated only by the first composite weight slice.
        t0_f, ns_f = CHUNKS[NCH - 1]
        xn_first = work.tile([128, 4, NT_F], FP16, name="xn", tag="xn", bufs=2)
        nc.sync.dma_start(xn_first[:, :, 0:ns_f * HW],
                          xn_d[:, :, t0_f * HW:t0_f * HW + ns_f * HW])

        wqc = cpool.tile([128, 4 * F], FP16, name="wqc", tag="wqc")
        wkc = cpool.tile([128, 4 * F], FP16, name="wkc", tag="wkc")
        for w_t, dr in ((wqc, w_d["wqc"]), (wkc, w_d["wkc"])):
            for k in range(4):
                eng = nc.sync if k % 2 == 0 else nc.scalar
                eng.dma_start(w_t[:, k * F:(k + 1) * F],
                              dr[:, k * F:(k + 1) * F])

        vec = gload("vecs", vecs_d, [128, 4 * NV], F32)
        vec4 = gload("vec4", vec4_d, [128, 24], F32)
        wvc = gload("wvc", w_d["wvc"], [128, 4 * F], FP16)
        posrep = gload("posrep", posrep_d, [D, HEADS * GS], FP16)
        blkm = gload("blkm", blk_d, [GS, NT_F], FP16)
        ones126 = gload("ones126", ones_d, [GS, 128], FP16)

        warm = cpool.tile([128, 1], F32, name="warm", tag="warm")
        for fn in (AF.Identity, AF.Relu, AF.Exp):
            nc.scalar.activation(warm[:], vec[:, 0:1], fn)
        # warmup collective: pays the CC setup cost and re-syncs core skew
        # before phase 1, so the real AR1 behaves like AR2 (~25us).
        arw_in = dpool.tile([128, 1], F32, name="arw_in", tag="arw_in")
        arw_out = dpool.tile([128, 1], F32, name="arw_out", tag="arw_out",
                             addr_space="Shared")
        nc.gpsimd.dma_start(arw_in[:], warm[:])
        nc.gpsimd.collective_compute(
            "AllReduce", ALU.add,
            replica_groups=[list(range(N_CORES))],
            ins=[arw_in.opt()], outs=[arw_out.opt()])
        arwb = work.tile([128, 1], F32, name="arwb", tag="arwb", bufs=1)
        nc.gpsimd.dma_start(arwb[:], arw_out[:])

        def vslice(k, name):
            i = VEC[name]
            return vec[:, k * NV + i:k * NV + i + 1]

        # stat-derived per-channel vectors (computed after all-reduces)
        sv = {}
        for nm in ["s2a", "t2a", "s3a", "t3a", "s3b", "t3b"]:
            sv[nm] = cpool.tile([128, 4], F32, name=nm, tag=nm)

        # incremental bn_stats buffers: one 6-wide slot per chunk per k-tile
        st3 = [cpool.tile([128, 6 * NCH], F32, name=f"st3_{k}", tag=f"st3_{k}")
               for k in range(4)]
        st7 = [cpool.tile([128, 6 * NCH], F32, name=f"st7_{k}", tag=f"st7_{k}")
               for k in range(4)]

        # persistent attention outputs (fp16, channel-major)
        o3_att = [apool.tile([128, T], FP16, name=f"o3att{k}", tag=f"o3att{k}")
                  for k in range(4)]
        o7_att = [apool.tile([128, T], FP16, name=f"o7att{k}", tag=f"o7att{k}")
                  for k in range(4)]

        def groups_of(ns):
            """(tok_off_in_chunk, gs_tokens) attention groups for ns samples"""
            out = []
            s = 0
            while s < ns:
                g = min(14, ns - s)
                out.append((s * HW, g * HW))
                s += g
            return out

        def wsl(wt, k, och):
            return wt[:, k * F + 128 * och:k * F + 128 * (och + 1)]

        def conv_gemm(wt, srcs, nt, och):
            """accumulate sum_k wt[k-slice,och].T @ srcs[k] into a fresh psum"""
            p = ps.tile([128, NT_F], F32, name="mmps", tag="mmps", bufs=4)
            for k in range(4):
                nc.tensor.matmul(p[:, :nt], wsl(wt, k, och), srcs[k][:, :nt],
                                 start=(k == 0), stop=(k == 3))
            return p

        def load_xn(xd, t0, nt, eng=None):
            """one chunk of pre-activated x: a single 3D-AP DMA"""
            xt = work.tile([128, 4, NT_F], FP16, name="xn", tag="xn", bufs=2)
            (eng or nc.sync).dma_start(xt[:, :, 0:nt], xd[:, :, t0:t0 + nt])
            return [xt[:, k, :] for k in range(4)]

        def mhsa(o3t, dest, t0, nt, ns, st, ci, q_on_scalar=True,
                 v_split=False, wq_t=None, wk_t=None, wv_t=None):
            """o3t: 4 input ch-tiles [128, nt] fp16; dest: 4 persistent fp16
            tiles, written at [:, t0:t0+nt]."""
            wq_t = wq_t if wq_t is not None else late_w['wq']
            wk_t = wk_t if wk_t is not None else late_w['wk']
            wv_t = wv_t if wv_t is not None else late_w['wv']
            grps = groups_of(ns)
            qs, ks_ = [], []
            for h in range(HEADS):
                p = conv_gemm(wq_t, o3t, nt, h)
                qh = work.tile([128, NT_F], FP16, name="qh", tag=f"qh{h}", bufs=1)
                if q_on_scalar:
                    nc.scalar.activation(qh[:, :nt], p[:, :nt], AF.Identity,
                                         bias=vslice(h, "qb"))
                else:
                    nc.vector.tensor_scalar(qh[:, :nt], p[:, :nt],
                                            vslice(h, "qb"), None, ALU.add)
                qs.append(qh)
                p = conv_gemm(wk_t, o3t, nt, h)
                kh = work.tile([128, NT_F], FP16, name="kh", tag=f"kh{h}", bufs=1)
                nc.scalar.activation(kh[:, :nt], p[:, :nt], AF.Identity,
                                     bias=vslice(h, "kb"))
                ks_.append(kh)
            vts = []
            for gi, (g0, gs) in enumerate(grps):
                p = ps.tile([GS, F], F32, name="mmps_v", tag="mmps", bufs=4)
                for k in range(4):
                    nc.tensor.matmul(p[:gs, :], o3t[k][:, g0:g0 + gs],
                                     wv_t[:, k * F:(k + 1) * F],
                                     start=(k == 0), stop=(k == 3))
                vt = work.tile([GS, F], FP16, name="vt", tag=f"vt{gi}", bufs=1)
                if v_split and gi % 2 == 1:
                    nc.vector.tensor_copy(vt[:gs, :], p[:gs, :])
                else:
                    nc.scalar.activation(vt[:gs, :], p[:gs, :], AF.Identity)
                vts.append(vt)
            gsmax = grps[0][1]
            for h in range(HEADS):
                L = ps.tile([GS, NT_F], F32, name="Lps", tag="L", bufs=2)
                for gi, (g0, gs) in enumerate(grps):
                    sl = slice(g0, g0 + gs)
                    nc.tensor.matmul(L[:gs, sl], ks_[h][:, sl], qs[h][:, sl],
                                     start=(gi == 0), stop=False)
                    nc.tensor.matmul(L[:gs, sl], qs[h][:, sl],
                                     posrep[:, GS * h:GS * h + gs],
                                     start=False,
                                     stop=(gi == len(grps) - 1))
                E = work.tile([GS, NT_F], FP16, name="E", tag="E")
                nc.scalar.activation(E[:gsmax, :nt], L[:gsmax, :nt], AF.Exp,
                                     bias=vec[:gsmax, VEC["c6"]:VEC["c6"] + 1])
                nc.vector.tensor_tensor(E[:gsmax, :nt], E[:gsmax, :nt],
                                        blkm[:gsmax, :nt], ALU.mult)
                Db = ps.tile([128, NT_F], F32, name="Dbps", tag="Db", bufs=1)
                nc.tensor.matmul(Db[:, :nt], ones126[:gsmax, :], E[:gsmax, :nt],
                                 start=True, stop=True)
                rcp = work.tile([128, NT_F], F32, name="rcp", tag="rcp",
                                bufs=1)
                nc.vector.reciprocal_approx_fast(rcp[:, :nt], Db[:, :nt])
                num = ps.tile([128, NT_F], F32, name="numps", tag="num", bufs=1)
                for gi, (g0, gs) in enumerate(grps):
                    sl = slice(g0, g0 + gs)
                    nc.tensor.matmul(num[:, sl], vts[gi][:gs, 128 * h:128 * (h + 1)],
                                     E[:gs, sl], start=True, stop=True)
                nc.vector.tensor_tensor(dest[h][:, t0 + 0:t0 + nt], num[:, :nt],
                                        rcp[:, :nt], ALU.mult)
                nc.vector.bn_stats(st[h][:, 6 * ci:6 * ci + 6],
                                   dest[h][:, t0:t0 + nt])

        def stats_kick_ar(st, ar_tag):
            """aggregate per-chunk bn stats (vector), kick the all-reduce.
            Readback is separate (ar_read) so AR-independent gpsimd DMAs can
            be emitted between kick and the queue-blocking readback."""
            arp = work.tile([128, 8], F32, name=f"arp{ar_tag}", tag="arp", bufs=1)
            for k in range(4):
                ag = work.tile([128, 2], F32, name="bnag", tag="bnag")
                nc.vector.bn_aggr(ag[:], st[k][:])
                nc.vector.tensor_scalar(arp[:, k:k + 1], ag[:, 0:1], float(T),
                                        None, ALU.mult)
                sq = work.tile([128, 1], F32, name="sq", tag="sq")
                nc.vector.tensor_tensor(sq[:], ag[:, 0:1], ag[:, 0:1], ALU.mult)
                nc.vector.tensor_tensor(sq[:], sq[:], ag[:, 1:2], ALU.add)
                nc.vector.tensor_scalar(arp[:, 4 + k:5 + k], sq[:], float(T),
                                        None, ALU.mult)
            ar_in = dpool.tile([128, 8], F32, name=f"ar_in{ar_tag}",
                               tag=f"ar_in{ar_tag}")
            ar_out = dpool.tile([128, 8], F32, name=f"ar_out{ar_tag}",
                                tag=f"ar_out{ar_tag}", addr_space="Shared")
            nc.gpsimd.dma_start(ar_in[:], arp[:])
            nc.gpsimd.collective_compute(
                "AllReduce", ALU.add,
                replica_groups=[list(range(N_CORES))],
                ins=[ar_in.opt()], outs=[ar_out.opt()])
            return ar_out

        def ar_read(ar_out, ar_tag):
            arr = work.tile([128, 8], F32, name=f"arr{ar_tag}", tag="arr", bufs=1)
            nc.gpsimd.dma_start(arr[:], ar_out[:])
            return arr

        def stats_post(arr, pairs):
            """derive (scale, shift) slabs [128,4] from AR sums, 4-wide.
            rsqrt via quake seed + 2 Newton steps, all on the vector engine
            (scalar Sqrt would trigger a 1.28us ACT_TABLE_LOAD stall).
            Emit only at a point where the all-reduce result is expected to
            have landed -- these ops sit in-order in both queues."""
            def tmp(nm):
                return work.tile([128, 4], F32, name=nm, tag=nm)
            mean = tmp("spm")
            nc.vector.tensor_scalar(mean[:], arr[:, 0:4], 1.0 / NTOT,
                                    None, ALU.mult)
            u = tmp("spu")
            nc.vector.tensor_scalar(u[:], arr[:, 4:8], 1.0 / NTOT,
                                    EPS, ALU.mult, ALU.add)
            msq = tmp("spq")
            nc.vector.tensor_tensor(msq[:], mean[:], mean[:], ALU.mult)
            nc.vector.tensor_tensor(u[:], u[:], msq[:], ALU.subtract)
            # quake rsqrt seed on int32-bitcast views
            y = tmp("spy")
            u_i = u[:].bitcast(mybir.dt.uint32)
            y_i = y[:].bitcast(mybir.dt.uint32)
            nc.vector.tensor_scalar(y_i, u_i, 1, None,
                                    ALU.logical_shift_right)
            nc.vector.tensor_scalar(y_i, y_i, 0, None, ALU.bitwise_not)
            # 0x5f3759df - x == ~x - (0xffffffff - 0x5f3759df); uint add
            # saturates on the DVE, so subtract (split to keep imms < 2^31)
            nc.vector.tensor_scalar(y_i, y_i, 0x50c8a620, None, ALU.subtract)
            nc.vector.tensor_scalar(y_i, y_i, 0x50000000, None, ALU.subtract)
            t_ = tmp("spt")
            for _ in range(1):
                nc.vector.tensor_tensor(t_[:], y[:], y[:], ALU.mult)
                nc.vector.tensor_tensor(t_[:], t_[:], u[:], ALU.mult)
                nc.vector.tensor_scalar(t_[:], t_[:], -0.5, 1.5,
                                        ALU.mult, ALU.add)
                nc.vector.tensor_tensor(y[:], y[:], t_[:], ALU.mult)
            for (s_t, t_t, gc, bc) in pairs:
                nc.vector.tensor_tensor(s_t[:], y[:], vec4[:, gc:gc + 4],
                                        ALU.mult)
                tm = tmp("sptm")
                nc.vector.tensor_tensor(tm[:], mean[:], s_t[:], ALU.mult)
                nc.vector.tensor_tensor(t_t[:], vec4[:, bc:bc + 4], tm[:],
                                        ALU.subtract)

        def att_act(att, t0, nt, s_t, t_t, tagp):
            """relu(s*att+t) for the 4 k-tiles of a chunk (scalar engine)"""
            outs = []
            for k in range(4):
                a = work.tile([128, NT_F], FP16, name=tagp, tag=f"{tagp}{k}",
                              bufs=2)
                nc.scalar.activation(a[:, :nt], att[k][:, t0:t0 + nt],
                                     AF.Relu, bias=t_t[:, k:k + 1],
                                     scale=s_t[:, k:k + 1])
                outs.append(a)
            return outs

        # ---------------- phase 1 ----------------
        # conv1 is folded into the composite q/k/v weights; each chunk goes
        # straight from the pre-activated x to the attention projections.
        late_w = {}
        xn_keep = {}
        p1_order = [NCH - 1] + list(range(NCH - 1))
        for pi, ci in enumerate(p1_order):
            s0, ns = CHUNKS[ci]
            if pi == 2:
                for n in ["w2x", "w3x", "w2a", "w3a", "w3b"]:
                    late_w[n] = gload(n, w_d[n], [128, 4 * F], FP16)
            t0, nt = s0 * HW, ns * HW
            if pi == 0:
                xn = [xn_first[:, k, :] for k in range(4)]
            elif pi == 2:
                # second full chunk split across gpsimd/scalar so it does not
                # serialize behind the weight slices during pipeline fill
                xt = work.tile([128, 4, NT_F], FP16, name="xn", tag="xn",
                               bufs=2)
                nc.gpsimd.dma_start(xt[:, 0:2, 0:nt], xn_d[:, 0:2, t0:t0 + nt])
                nc.scalar.dma_start(xt[:, 2:4, 0:nt], xn_d[:, 2:4, t0:t0 + nt])
                xn = [xt[:, k, :] for k in range(4)]
            else:
                xn = load_xn(xn_d, t0, nt)
            if ci in (NCH - 2, NCH - 3):
                xn_keep[ci] = xn   # still resident for the AR1 window
            mhsa(xn, o3_att, t0, nt, ns, st3, ci, v_split=True,
                 wq_t=wqc, wk_t=wkc, wv_t=wvc)
            if pi <= 1:
                # keep the HAM warm across the early DMA-wait gaps (the
                # first chunks' matmuls are sparse while weights stream in)
                for wg in range(4 if pi == 0 else 2):
                    fp_ = ps.tile([128, NT_F], F32, name="fillw",
                                  tag="mmps", bufs=4)
                    nc.tensor.matmul(fp_[:, :NT_F], wrm[:, 0:128],
                                     wrm[:, 128:632], start=True, stop=True)

        # kick AR1 first (its latency is the phase-2 critical path), THEN the
        # raw q/k/v weight reuse DMAs (transfers run during the AR; the
        # composite tiles' SBUF is dead after phase 1 and the framework
        # serializes the DMA behind phase 1's last reads), then the
        # queue-blocking readback.
        ar1_out = stats_kick_ar(st3, "1")
        for n, tg in (("wq", "wqc"), ("wk", "wkc"), ("wv", "wvc")):
            t_ = cpool.tile([128, 4 * F], FP16, name=n, tag=tg)
            nc.gpsimd.dma_start(t_[:], w_d[n][:])
            late_w[n] = t_
        arr1 = ar_read(ar1_out, "1")

        # fill the AR latency with AR-independent x-half convs: conv2x over
        # all chunks, interleaved with conv3x for the EARLY_SET chunks
        # (all-scalar conv3x evacs so nothing sits behind the vector queue
        # once stats_post blocks it).  The first two chunks reuse the x
        # tiles still resident from phase 1, so the window starts DMA-free.
        part2 = {}
        part3 = {}
        ld_engs = [nc.sync, nc.scalar]

        def filler(n_grp):
            # keep the PE streaming across the AR boundary so the HAM does
            # not re-throttle; results are discarded.  small groups so real
            # work enqueued later is not stuck behind a long fake stream.
            for fg in range(n_grp):
                fp_ = ps.tile([128, NT_F], F32, name="fill", tag="mmps",
                              bufs=4)
                for k in range(2):
                    nc.tensor.matmul(fp_[:, :NT_F],
                                     wsl(late_w['w2x'], k, fg % 4),
                                     late_w['w2x'][:, 0:NT_F],
                                     start=(k == 0), stop=(k == 1))

        for wi, ci in enumerate(WIN_ORDER):
            s0, ns = CHUNKS[ci]
            t0, nt = s0 * HW, ns * HW
            if ci in xn_keep:
                xn = xn_keep.pop(ci)
            else:
                xn = load_xn(xn_d, t0, nt, eng=ld_engs[wi % 2])
            if ci in WIN_ORDER[-2:]:
                xn_keep[ci] = xn   # still resident for the AR2 window
            for o in range(4):
                p = conv_gemm(late_w['w2x'], xn, nt, o)
                pt = work.tile([128, nt], FP16, name="part2",
                               tag=f"part{ci}_{o}", bufs=1)
                if o % 2 == 0:
                    nc.scalar.activation(pt[:, :nt], p[:, :nt], AF.Identity)
                else:
                    nc.vector.tensor_copy(pt[:, :nt], p[:, :nt])
                part2[(ci, o)] = pt
            if ci in EARLY_SET:
                for o in range(4):
                    p = conv_gemm(late_w['w3x'], xn, nt, o)
                    pt = work.tile([128, nt], FP16, name="part3e",
                                   tag=f"p3e{ci}_{o}", bufs=1)
                    nc.scalar.activation(pt[:, :nt], p[:, :nt], AF.Identity)
                    part3[(ci, o)] = pt
        filler(4)
        stats_post(arr1, [(sv["s2a"], sv["t2a"], 0, 4),
                          (sv["s3a"], sv["t3a"], 8, 12)])
        filler(3)

        # ---------------- phase 2 ----------------
        for ci, (s0, ns) in enumerate(CHUNKS):
            t0, nt = s0 * HW, ns * HW
            o3a = att_act(o3_att, t0, nt, sv["s2a"], sv["t2a"], "oa")
            o7 = []
            for o in range(4):
                p = conv_gemm(late_w['w2a'], o3a, nt, o)
                o7k = work.tile([128, NT_F], FP16, name="o7", tag=f"o3{o}")
                nc.vector.scalar_tensor_tensor(
                    o7k[:, :nt], p[:, :nt], vslice(o, "b2"),
                    part2[(ci, o)][:, :nt], ALU.add, ALU.add)
                o7.append(o7k)
            mhsa(o7, o7_att, t0, nt, ns, st7, ci, q_on_scalar=True)

        # kick AR2; fill with the remaining conv3 x+a work.  stats_post(arr2)
        # is emitted after the loop so its (blocked) vector ops cannot gate
        # the loop's own psum evacuations.
        ar2_out = stats_kick_ar(st7, "2")
        arr2 = ar_read(ar2_out, "2")
        for wi, ci in enumerate(AR2WIN_ORDER):
            s0, ns = CHUNKS[ci]
            t0, nt = s0 * HW, ns * HW
            o3a = att_act(o3_att, t0, nt, sv["s3a"], sv["t3a"], "oa")
            if ci in EARLY_SET:
                # a-half only; add in place into the stored x-half partial
                for o in range(4):
                    p = conv_gemm(late_w['w3a'], o3a, nt, o)
                    pt = part3[(ci, o)]
                    nc.vector.tensor_tensor(pt[:, :nt], p[:, :nt],
                                            pt[:, :nt], ALU.add)
            else:
                if ci in xn_keep:
                    xn = xn_keep.pop(ci)
                else:
                    xn = load_xn(xn_d, t0, nt, eng=ld_engs[wi % 2])
                for o in range(4):
                    p = ps.tile([128, NT_F], F32, name="mmps3", tag="mmps",
                                bufs=4)
                    for k in range(4):
                        nc.tensor.matmul(p[:, :nt], wsl(late_w['w3x'], k, o),
                                         xn[k][:, :nt], start=(k == 0),
                                         stop=False)
                    for k in range(4):
                        nc.tensor.matmul(p[:, :nt], wsl(late_w['w3a'], k, o),
                                         o3a[k][:, :nt], start=False,
                                         stop=(k == 3))
                    pt = work.tile([128, nt], FP16, name="part3",
                                   tag=f"part{ci}_{o}", bufs=1)
                    if o % 2 == 0 or wi >= len(AR2WIN_ORDER) - 2:
                        nc.scalar.activation(pt[:, :nt], p[:, :nt], AF.Identity)
                    else:
                        nc.vector.tensor_copy(pt[:, :nt], p[:, :nt])
                    part3[(ci, o)] = pt
        stats_post(arr2, [(sv["s3b"], sv["t3b"], 16, 20)])
        filler(2)

        # ---------------- phase 3 ----------------
        # natural order: the kernel ends on the small remainder chunk, whose
        # output DMA drains in ~1us (full chunks spread over three queues).
        out_engs = [nc.gpsimd, nc.sync, nc.scalar]
        for pi, ci in enumerate(range(NCH)):
            s0, ns = CHUNKS[ci]
            t0, nt = s0 * HW, ns * HW
            o7a = att_act(o7_att, t0, nt, sv["s3b"], sv["t3b"], "oa")
            for o in range(4):
                p = conv_gemm(late_w['w3b'], o7a, nt, o)
                ot = work.tile([128, NT_F], FP16, name="ot", tag=f"ot{o}",
                               bufs=1)
                nc.vector.scalar_tensor_tensor(
                    ot[:, :nt], p[:, :nt], vslice(o, "b3"),
                    part3[(ci, o)][:, :nt], ALU.add, ALU.add)
                out_engs[(pi * 4 + o) % 3].dma_start(out_d[:, o, t0:t0 + nt],
                                                     ot[:, :nt])
        es.close()

    nc.compile()
    return nc


def _host_prep(inputs):
    g = {k: np.asarray(v, np.float32) for k, v in inputs.items()}
    x = g["x"]
    m = x.mean(axis=(0, 2, 3))
    v = x.var(axis=(0, 2, 3))
    rs = 1.0 / np.sqrt(v + EPS)

    def st(gam, bet):
        s = gam * rs
        return s, bet - m * s

    hf = np.float16

    def xn_prep(gam, bet):
        s, t = st(gam, bet)
        xa = np.maximum(x * s[None, :, None, None] + t[None, :, None, None],
                        0.0).astype(hf)
        # [B, C, 3, 3] -> [C, B, HW] -> per-core [128, 4, T]
        return np.ascontiguousarray(xa.reshape(B, C, HW).transpose(1, 0, 2))

    # the kernel shares one pre-activated x across all three phases; valid
    # because the three BNs carry identical gamma/beta on the x channels.
    assert np.array_equal(g["bn2_g"][:C], g["bn1_g"])
    assert np.array_equal(g["bn2_b"][:C], g["bn1_b"])
    assert np.array_equal(g["bn3_g"][:C], g["bn1_g"])
    assert np.array_equal(g["bn3_b"][:C], g["bn1_b"])
    xn1 = xn_prep(g["bn1_g"], g["bn1_b"])

    # composite MHSA1 projection weights: q = (Wq W1) a1 + (Wq b1 + qb)
    wq_c = g["q_w"] @ g["w1"]
    wk_c = g["k_w"] @ g["w1"]
    wv_c = g["v_w"] @ g["w1"]
    qb_c = g["q_w"] @ g["b1"] + g["q_b"]
    kb_c = g["k_w"] @ g["b1"] + g["k_b"]
    # v bias (and the folded W1 bias through v) cancels in BN downstream.

    vec_cols = {}
    vec_cols["qb"] = qb_c
    vec_cols["kb"] = kb_c
    vec_cols["b2"] = g["b2"]
    vec_cols["b3"] = g["b3"]
    vec_cols["g2a"] = g["bn2_g"][C:]
    vec_cols["b2a"] = g["bn2_b"][C:]
    vec_cols["g3a"] = g["bn3_g"][C:2 * C]
    vec_cols["b3a"] = g["bn3_b"][C:2 * C]
    vec_cols["g3b"] = g["bn3_g"][2 * C:]
    vec_cols["b3b"] = g["bn3_b"][2 * C:]
    vec_cols["c6"] = np.full(C, -9.0, np.float32)
    vecs = np.zeros((128, 4, NV), np.float32)
    for n, i in VEC.items():
        vecs[:, :, i] = vec_cols[n].reshape(4, 128).T
    vec4 = np.zeros((128, 24), np.float32)
    for j, n in enumerate(["g2a", "b2a", "g3a", "b3a", "g3b", "b3b"]):
        vec4[:, 4 * j:4 * j + 4] = vec_cols[n].reshape(4, 128).T

    def wtile(wT):
        # [C_in_512, F] -> [128, 4*F]
        return np.ascontiguousarray(
            wT.reshape(4, 128, F).transpose(1, 0, 2).reshape(128, 4 * F)
        ).astype(hf)

    pos = (g["rel_h"] + g["rel_w"]).reshape(HEADS, D, HW)
    posrep = np.tile(pos, (1, 1, 14)).transpose(1, 0, 2).reshape(D, HEADS * GS)

    b_of = np.repeat(np.arange(14), HW)
    blk1 = (b_of[:, None] == b_of[None, :]).astype(np.float32)

    shared = {
        "wqc": wtile(wq_c.T),
        "wkc": wtile(wk_c.T),
        "wvc": wtile(wv_c.T),
        "wq": wtile(g["q_w"].T),
        "wk": wtile(g["k_w"].T),
        "wv": wtile(g["v_w"].T),
        "w2x": wtile(g["w2"].T[:C]),
        "w2a": wtile(g["w2"].T[C:]),
        "w3x": wtile(g["w3"].T[:C]),
        "w3a": wtile(g["w3"].T[C:2 * C]),
        "w3b": wtile(g["w3"].T[2 * C:]),
        "vecs": vecs.reshape(128, 4 * NV),
        "vec4": vec4,
        "posrep": posrep.astype(hf),
        "blkmask": np.tile(blk1, (1, 4)).astype(hf),
        "ones126": np.ones((GS, 128), np.float32).astype(hf),
    }
    in_maps = []
    for c in range(N_CORES):
        def core_x(xa):
            xs = xa[:, BC * c:BC * (c + 1), :].reshape(C, T)
            return np.ascontiguousarray(
                xs.reshape(4, 128, T).transpose(1, 0, 2))
        in_maps.append(dict(shared, xn=core_x(xn1)))
    return in_maps


def kernel(**inputs):
    if "nc" not in _cache:
        _cache["nc"] = _build()
    nc = _cache["nc"]
    in_maps = _host_prep(inputs)
    res = run_bass_kernel_spmd(nc, in_maps, core_ids=list(range(N_CORES)))
    parts = [res.results[c]["out_cm"].astype(np.float32)
             .reshape(128, 4, BC, HW).transpose(1, 0, 2, 3).reshape(F, BC, HW)
             for c in range(N_CORES)]
    full = np.concatenate(parts, axis=1)          # [F, B, HW]
    return np.ascontiguousarray(full.transpose(1, 0, 2)).reshape(B, F, 3, 3)


# revision 50
# speedup vs baseline: 1.1202x; 1.1202x over previous
"""Trainium2 Bass kernel for nn_DenseBlock_MHSA (dense_cnn).

Data-parallel over batch across 8 NeuronCores (512 samples/core).
Layout: channel-major activations [ch, tokens] on chip, tokens = 512*9 = 4608/core.
All GEMMs run fp16 (full PE rate), accumulation fp32 in PSUM. Per-sample 9x9
attention is batched 14 samples/group as masked 126x126 blocks; the softmax
mask bakes in a -9 logit shift so fp16 exp cannot overflow.

Optimizations over the 588-608us baseline (measured ~542us, rel err 1.4e-3;
fp8/DoubleRow was evaluated and rejected: e4m3's ~3.6%/operand quantization
noise gives 3.5-10% final error vs the 2e-2 gate, and exact hi+lo
compensation needs more DoubleRow passes than plain fp16 streaming):
- conv1 folded into the MHSA1 q/k/v projections: o3 = W1 a1 is consumed only
  by the first attention's channel GEMMs, so q = (Wq W1) a1 etc. with
  host-composited weights. Removes one full 512x512 GEMM unit (~38us of PE
  streaming) plus the o3 psum evacuations.
- bn1/bn2/bn3 carry identical gamma/beta on the x channels, so the
  pre-activated x is ONE tensor shared by all three phases: the conv2x and
  conv3x window convs share a single load per chunk, and chunk tiles still
  resident from the previous phase seed each window loop (the AR1 window
  starts DMA-free, halving its DMA pressure).
- AR seams: the all-reduce is kicked before the (queue-blocking) result
  readback and the raw q/k/v weight reloads (which reuse the dead composite
  tiles' SBUF); stats_post runs after the window work so its blocked vector
  ops cannot gate psum evacuations; small keep-warm PE fillers bridge the
  residual cross-core-skew dead zone so phase 2 doesn't restart at HAM 4/8.
- stats_post rsqrt on the vector engine (quake seed on uint32-bitcast views
  + 1 Newton step; uint add saturates on the DVE so the magic constant is
  applied via bitwise_not + two subtracts) instead of scalar Sqrt, whose
  ACT_TABLE_LOAD cost 4x 1.28us mid-kernel scalar stalls at the seams.
- startup: composite q/k weights k-sliced alternating sync/scalar queues,
  first (small) x chunk on gpsimd ahead of the consts, second full chunk on
  scalar; output DMAs rotate per-tile over three queues and phase 3 ends on
  the small remainder chunk so the final drain is ~1us.
- psum evacuations balanced across Scalar and Vector per phase; L psum
  double-buffered so head h+1 scores overlap head h softmax; relu'd
  attention inputs double-buffered.
- v-bias dropped: BN is shift-invariant, so the conv-v bias cancels exactly
  in every consumer of the attention outputs (same for the folded W1 bias
  through v).
"""

import numpy as np

import concourse.bass as bass
import concourse.mybir as mybir
import concourse.tile as tile
from concourse import bacc
from concourse.bass_utils import run_bass_kernel_spmd

F32 = mybir.dt.float32
FP16 = mybir.dt.float16
AF = mybir.ActivationFunctionType
ALU = mybir.AluOpType

N_CORES = 8
B, C, F, HW = 4096, 512, 512, 9
HEADS, D = 4, 128
BC = B // N_CORES            # samples per core
T = BC * HW                  # tokens per core
EPS = 1e-5
NTOT = float(B * HW)         # global token count for BN stats

CH_S = 56                    # samples per chunk
NT_F = CH_S * HW             # 504 tokens per full chunk
GS = 14 * HW                 # 126 tokens per attention group
CHUNKS = [(c * CH_S, CH_S) for c in range(BC // CH_S)]
if BC % CH_S:
    CHUNKS.append(((BC // CH_S) * CH_S, BC % CH_S))
NCH = len(CHUNKS)
EARLY_SET = (8, 7, 0, 1)     # chunks whose conv3 x-half runs in the AR1 window
WIN_ORDER = (8, 7, 0, 1, 2, 3, 4, 5, 9, 6)    # conv2x/conv3x window order
AR2WIN_ORDER = (9, 6, 8, 7, 0, 1, 2, 3, 4, 5)  # conv3 a-half (+x) order

VEC = {n: i for i, n in enumerate(
    ["qb", "kb", "b2", "b3",
     "g2a", "b2a", "g3a", "b3a", "g3b", "b3b", "c6"])}
NV = len(VEC)

_cache = {}


def _build():
    nc = bacc.Bacc("TRN2", target_bir_lowering=False, debug=False,
                   num_devices=N_CORES)

    dram = {}

    def din(name, shape, dt):
        dram[name] = nc.dram_tensor(name, shape, dt, kind="ExternalInput").ap()
        return dram[name]

    # bn1/bn2/bn3 share gamma=1, beta=0 on the x channels, so the
    # pre-activated x is one tensor shared by all three phases.
    xn_d = din("xn", [128, 4, T], FP16)
    w_d = {n: din(n, [128, 4 * F], FP16)
           for n in ["wqc", "wkc", "wvc", "wq", "wk", "wv",
                     "w2x", "w2a", "w3x", "w3a", "w3b"]}
    vecs_d = din("vecs", [128, 4 * NV], F32)
    vec4_d = din("vec4", [128, 24], F32)
    posrep_d = din("posrep", [D, HEADS * GS], FP16)
    blk_d = din("blkmask", [GS, NT_F], FP16)
    ones_d = din("ones126", [GS, 128], FP16)
    out_d = nc.dram_tensor("out_cm", [128, 4, T], FP16, kind="ExternalOutput").ap()

    with tile.TileContext(nc) as tc:
        from contextlib import ExitStack
        es = ExitStack()
        cpool = es.enter_context(tc.tile_pool(name="consts", bufs=1))
        apool = es.enter_context(tc.tile_pool(name="attres", bufs=1))
        dpool = es.enter_context(tc.tile_pool(name="dram", bufs=1, space="DRAM"))
        work = es.enter_context(tc.tile_pool(name="work", bufs=2))
        ps = es.enter_context(tc.tile_pool(name="ps", bufs=2, space="PSUM"))

        # ---- startup-critical loads.  sync: composite q weights (first
        # GEMM), scalar/vector queues: composite k/v weights, gpsimd: first
        # (small) x chunk then small consts.  Raw + phase-2/3 weights are
        # DMA'd later, once phase 1 is underway. ----
        def gload(name, dr, shape, dt, eng=None):
            t_ = cpool.tile(shape, dt, name=name, tag=name)
            (eng or nc.gpsimd).dma_start(t_[:], dr[:])
            return t_

        # PE pre-warm: the HAM holds the PE at 4/8 (1.2 GHz) until it sees
        # ~3.4us of sustained activity, so without this the first ~20us of
        # real matmuls run at half clock.  Stream zero matmuls from a
        # memset tile while the startup DMAs are still in flight.
        wrm = cpool.tile([128, 640], FP16, name="wrm", tag="wrm")
        nc.vector.memset(wrm[:], 0.0)
        for wg in range(10):
            fp_ = ps.tile([128, NT_F], F32, name="fillw", tag="mmps", bufs=4)
            nc.tensor.matmul(fp_[:, :NT_F], wrm[:, 0:128], wrm[:, 128:632],
                             start=True, stop=True)

        # first chunk of phase 1 is the small remainder chunk: its load goes
        # first on sync (gpsimd's first DMA pays ring warmup) so the first
        # real matmul is gated only by the first composite weight slice.
        t0_f, ns_f = CHUNKS[NCH - 1]
        xn_first = work.tile([128, 4, NT_F], FP16, name="xn", tag="xn", bufs=2)
        nc.sync.dma_start(xn_first[:, :, 0:ns_f * HW],
                          xn_d[:, :, t0_f * HW:t0_f * HW + ns_f * HW])

        wqc = cpool.tile([128, 4 * F], FP16, name="wqc", tag="wqc")
        wkc = cpool.tile([128, 4 * F], FP16, name="wkc", tag="wkc")
        for w_t, dr in ((wqc, w_d["wqc"]), (wkc, w_d["wkc"])):
            for k in range(4):
                eng = nc.sync if k % 2 == 0 else nc.scalar
                eng.dma_start(w_t[:, k * F:(k + 1) * F],
                              dr[:, k * F:(k + 1) * F])

        vec = gload("vecs", vecs_d, [128, 4 * NV], F32)
        vec4 = gload("vec4", vec4_d, [128, 24], F32)
        wvc = gload("wvc", w_d["wvc"], [128, 4 * F], FP16)
        posrep = gload("posrep", posrep_d, [D, HEADS * GS], FP16)
        blkm = gload("blkm", blk_d, [GS, NT_F], FP16)
        ones126 = gload("ones126", ones_d, [GS, 128], FP16)

        warm = cpool.tile([128, 1], F32, name="warm", tag="warm")
        for fn in (AF.Identity, AF.Relu, AF.Exp):
            nc.scalar.activation(warm[:], vec[:, 0:1], fn)
        # warmup collective: pays the CC setup cost and re-syncs core skew
        # before phase 1, so the real AR1 behaves like AR2 (~25us).
        arw_in = dpool.tile([128, 1], F32, name="arw_in", tag="arw_in")
        arw_out = dpool.tile([128, 1], F32, name="arw_out", tag="arw_out",
                             addr_space="Shared")
        nc.gpsimd.dma_start(arw_in[:], warm[:])
        nc.gpsimd.collective_compute(
            "AllReduce", ALU.add,
            replica_groups=[list(range(N_CORES))],
            ins=[arw_in.opt()], outs=[arw_out.opt()])
        arwb = work.tile([128, 1], F32, name="arwb", tag="arwb", bufs=1)
        nc.gpsimd.dma_start(arwb[:], arw_out[:])

        def vslice(k, name):
            i = VEC[name]
            return vec[:, k * NV + i:k * NV + i + 1]

        # stat-derived per-channel vectors (computed after all-reduces)
        sv = {}
        for nm in ["s2a", "t2a", "s3a", "t3a", "s3b", "t3b"]:
            sv[nm] = cpool.tile([128, 4], F32, name=nm, tag=nm)

        # incremental bn_stats buffers: one 6-wide slot per chunk per k-tile
        st3 = [cpool.tile([128, 6 * NCH], F32, name=f"st3_{k}", tag=f"st3_{k}")
               for k in range(4)]
        st7 = [cpool.tile([128, 6 * NCH], F32, name=f"st7_{k}", tag=f"st7_{k}")
               for k in range(4)]

        # persistent attention outputs (fp16, channel-major)
        o3_att = [apool.tile([128, T], FP16, name=f"o3att{k}", tag=f"o3att{k}")
                  for k in range(4)]
        o7_att = [apool.tile([128, T], FP16, name=f"o7att{k}", tag=f"o7att{k}")
                  for k in range(4)]

        def groups_of(ns):
            """(tok_off_in_chunk, gs_tokens) attention groups for ns samples"""
            out = []
            s = 0
            while s < ns:
                g = min(14, ns - s)
                out.append((s * HW, g * HW))
                s += g
            return out

        def wsl(wt, k, och):
            return wt[:, k * F + 128 * och:k * F + 128 * (och + 1)]

        def conv_gemm(wt, srcs, nt, och):
            """accumulate sum_k wt[k-slice,och].T @ srcs[k] into a fresh psum"""
            p = ps.tile([128, NT_F], F32, name="mmps", tag="mmps", bufs=4)
            for k in range(4):
                nc.tensor.matmul(p[:, :nt], wsl(wt, k, och), srcs[k][:, :nt],
                                 start=(k == 0), stop=(k == 3))
            return p

        def load_xn(xd, t0, nt, eng=None):
            """one chunk of pre-activated x: a single 3D-AP DMA"""
            xt = work.tile([128, 4, NT_F], FP16, name="xn", tag="xn", bufs=2)
            (eng or nc.sync).dma_start(xt[:, :, 0:nt], xd[:, :, t0:t0 + nt])
            return [xt[:, k, :] for k in range(4)]

        def mhsa(o3t, dest, t0, nt, ns, st, ci, q_on_scalar=True,
                 v_split=False, wq_t=None, wk_t=None, wv_t=None):
            """o3t: 4 input ch-tiles [128, nt] fp16; dest: 4 persistent fp16
            tiles, written at [:, t0:t0+nt]."""
            wq_t = wq_t if wq_t is not None else late_w['wq']
            wk_t = wk_t if wk_t is not None else late_w['wk']
            wv_t = wv_t if wv_t is not None else late_w['wv']
            grps = groups_of(ns)
            qs, ks_ = [], []
            for h in range(HEADS):
                p = conv_gemm(wq_t, o3t, nt, h)
                qh = work.tile([128, NT_F], FP16, name="qh", tag=f"qh{h}", bufs=1)
                if q_on_scalar:
                    nc.scalar.activation(qh[:, :nt], p[:, :nt], AF.Identity,
                                         bias=vslice(h, "qb"))
                else:
                    nc.vector.tensor_scalar(qh[:, :nt], p[:, :nt],
                                            vslice(h, "qb"), None, ALU.add)
                qs.append(qh)
                p = conv_gemm(wk_t, o3t, nt, h)
                kh = work.tile([128, NT_F], FP16, name="kh", tag=f"kh{h}", bufs=1)
                nc.scalar.activation(kh[:, :nt], p[:, :nt], AF.Identity,
                                     bias=vslice(h, "kb"))
                ks_.append(kh)
            vts = []
            for gi, (g0, gs) in enumerate(grps):
                p = ps.tile([GS, F], F32, name="mmps_v", tag="mmps", bufs=4)
                for k in range(4):
                    nc.tensor.matmul(p[:gs, :], o3t[k][:, g0:g0 + gs],
                                     wv_t[:, k * F:(k + 1) * F],
                                     start=(k == 0), stop=(k == 3))
                vt = work.tile([GS, F], FP16, name="vt", tag=f"vt{gi}", bufs=1)
                if v_split and gi % 2 == 1:
                    nc.vector.tensor_copy(vt[:gs, :], p[:gs, :])
                else:
                    nc.scalar.activation(vt[:gs, :], p[:gs, :], AF.Identity)
                vts.append(vt)
            gsmax = grps[0][1]
            for h in range(HEADS):
                L = ps.tile([GS, NT_F], F32, name="Lps", tag="L", bufs=2)
                for gi, (g0, gs) in enumerate(grps):
                    sl = slice(g0, g0 + gs)
                    nc.tensor.matmul(L[:gs, sl], ks_[h][:, sl], qs[h][:, sl],
                                     start=(gi == 0), stop=False)
                    nc.tensor.matmul(L[:gs, sl], qs[h][:, sl],
                                     posrep[:, GS * h:GS * h + gs],
                                     start=False,
                                     stop=(gi == len(grps) - 1))
                E = work.tile([GS, NT_F], FP16, name="E", tag="E")
                nc.scalar.activation(E[:gsmax, :nt], L[:gsmax, :nt], AF.Exp,
                                     bias=vec[:gsmax, VEC["c6"]:VEC["c6"] + 1])
                nc.vector.tensor_tensor(E[:gsmax, :nt], E[:gsmax, :nt],
                                        blkm[:gsmax, :nt], ALU.mult)
                Db = ps.tile([128, NT_F], F32, name="Dbps", tag="Db", bufs=1)
                nc.tensor.matmul(Db[:, :nt], ones126[:gsmax, :], E[:gsmax, :nt],
                                 start=True, stop=True)
                rcp = work.tile([128, NT_F], F32, name="rcp", tag="rcp",
                                bufs=1)
                nc.vector.reciprocal_approx_fast(rcp[:, :nt], Db[:, :nt])
                num = ps.tile([128, NT_F], F32, name="numps", tag="num", bufs=1)
                for gi, (g0, gs) in enumerate(grps):
                    sl = slice(g0, g0 + gs)
                    nc.tensor.matmul(num[:, sl], vts[gi][:gs, 128 * h:128 * (h + 1)],
                                     E[:gs, sl], start=True, stop=True)
                nc.vector.tensor_tensor(dest[h][:, t0 + 0:t0 + nt], num[:, :nt],
                                        rcp[:, :nt], ALU.mult)
                nc.vector.bn_stats(st[h][:, 6 * ci:6 * ci + 6],
                                   dest[h][:, t0:t0 + nt])

        def stats_kick_ar(st, ar_tag):
            """aggregate per-chunk bn stats (vector), kick the all-reduce.
            Readback is separate (ar_read) so AR-independent gpsimd DMAs can
            be emitted between kick and the queue-blocking readback."""
            arp = work.tile([128, 8], F32, name=f"arp{ar_tag}", tag="arp", bufs=1)
            for k in range(4):
                ag = work.tile([128, 2], F32, name="bnag", tag="bnag")
                nc.vector.bn_aggr(ag[:], st[k][:])
                nc.vector.tensor_scalar(arp[:, k:k + 1], ag[:, 0:1], float(T),
                                        None, ALU.mult)
                sq = work.tile([128, 1], F32, name="sq", tag="sq")
                nc.vector.tensor_tensor(sq[:], ag[:, 0:1], ag[:, 0:1], ALU.mult)
                nc.vector.tensor_tensor(sq[:], sq[:], ag[:, 1:2], ALU.add)
                nc.vector.tensor_scalar(arp[:, 4 + k:5 + k], sq[:], float(T),
                                        None, ALU.mult)
            ar_in = dpool.tile([128, 8], F32, name=f"ar_in{ar_tag}",
                               tag=f"ar_in{ar_tag}")
            ar_out = dpool.tile([128, 8], F32, name=f"ar_out{ar_tag}",
                                tag=f"ar_out{ar_tag}", addr_space="Shared")
            nc.gpsimd.dma_start(ar_in[:], arp[:])
            nc.gpsimd.collective_compute(
                "AllReduce", ALU.add,
                replica_groups=[list(range(N_CORES))],
                ins=[ar_in.opt()], outs=[ar_out.opt()])
            return ar_out

        def ar_read(ar_out, ar_tag):
            arr = work.tile([128, 8], F32, name=f"arr{ar_tag}", tag="arr", bufs=1)
            nc.gpsimd.dma_start(arr[:], ar_out[:])
            return arr

        def stats_post(arr, pairs):
            """derive (scale, shift) slabs [128,4] from AR sums, 4-wide.
            rsqrt via quake seed + 2 Newton steps, all on the vector engine
            (scalar Sqrt would trigger a 1.28us ACT_TABLE_LOAD stall).
            Emit only at a point where the all-reduce result is expected to
            have landed -- these ops sit in-order in both queues."""
            def tmp(nm):
                return work.tile([128, 4], F32, name=nm, tag=nm)
            mean = tmp("spm")
            nc.vector.tensor_scalar(mean[:], arr[:, 0:4], 1.0 / NTOT,
                                    None, ALU.mult)
            u = tmp("spu")
            nc.vector.tensor_scalar(u[:], arr[:, 4:8], 1.0 / NTOT,
                                    EPS, ALU.mult, ALU.add)
            msq = tmp("spq")
            nc.vector.tensor_tensor(msq[:], mean[:], mean[:], ALU.mult)
            nc.vector.tensor_tensor(u[:], u[:], msq[:], ALU.subtract)
            # quake rsqrt seed on int32-bitcast views
            y = tmp("spy")
            u_i = u[:].bitcast(mybir.dt.uint32)
            y_i = y[:].bitcast(mybir.dt.uint32)
            nc.vector.tensor_scalar(y_i, u_i, 1, None,
                                    ALU.logical_shift_right)
            nc.vector.tensor_scalar(y_i, y_i, 0, None, ALU.bitwise_not)
            # 0x5f3759df - x == ~x - (0xffffffff - 0x5f3759df); uint add
            # saturates on the DVE, so subtract (split to keep imms < 2^31)
            nc.vector.tensor_scalar(y_i, y_i, 0x50c8a620, None, ALU.subtract)
            nc.vector.tensor_scalar(y_i, y_i, 0x50000000, None, ALU.subtract)
            t_ = tmp("spt")
            for _ in range(1):
                nc.vector.tensor_tensor(t_[:], y[:], y[:], ALU.mult)
                nc.vector.tensor_tensor(t_[:], t_[:], u[:], ALU.mult)
                nc.vector.tensor_scalar(t_[:], t_[:], -0.5, 1.5,
                                        ALU.mult, ALU.add)
                nc.vector.tensor_tensor(y[:], y[:], t_[:], ALU.mult)
            for (s_t, t_t, gc, bc) in pairs:
                nc.vector.tensor_tensor(s_t[:], y[:], vec4[:, gc:gc + 4],
                                        ALU.mult)
                tm = tmp("sptm")
                nc.vector.tensor_tensor(tm[:], mean[:], s_t[:], ALU.mult)
                nc.vector.tensor_tensor(t_t[:], vec4[:, bc:bc + 4], tm[:],
                                        ALU.subtract)

        def att_act(att, t0, nt, s_t, t_t, tagp):
            """relu(s*att+t) for the 4 k-tiles of a chunk (scalar engine)"""
            outs = []
            for k in range(4):
                a = work.tile([128, NT_F], FP16, name=tagp, tag=f"{tagp}{k}",
                              bufs=2)
                nc.scalar.activation(a[:, :nt], att[k][:, t0:t0 + nt],
                                     AF.Relu, bias=t_t[:, k:k + 1],
                                     scale=s_t[:, k:k + 1])
                outs.append(a)
            return outs

        # ---------------- phase 1 ----------------
        # conv1 is folded into the composite q/k/v weights; each chunk goes
        # straight from the pre-activated x to the attention projections.
        late_w = {}
        xn_keep = {}
        p1_order = [NCH - 1] + list(range(NCH - 1))
        for pi, ci in enumerate(p1_order):
            s0, ns = CHUNKS[ci]
            if pi == 2:
                for n in ["w2x", "w3x", "w2a", "w3a", "w3b"]:
                    late_w[n] = gload(n, w_d[n], [128, 4 * F], FP16)
            t0, nt = s0 * HW, ns * HW
            if pi == 0:
                xn = [xn_first[:, k, :] for k in range(4)]
            else:
                # second full chunk on the scalar queue so the pipeline-fill
                # loads overlap instead of serializing on sync (gpsimd is NOT
                # usable here: its queue blocks on the warmup-AR readback)
                xn = load_xn(xn_d, t0, nt,
                             eng=nc.scalar if pi == 2 else nc.sync)
            if ci in (NCH - 2, NCH - 3):
                xn_keep[ci] = xn   # still resident for the AR1 window
            mhsa(xn, o3_att, t0, nt, ns, st3, ci, v_split=True,
                 wq_t=wqc, wk_t=wkc, wv_t=wvc)
            if pi <= 1:
                # keep the HAM warm across the early DMA-wait gaps (the
                # first chunks' matmuls are sparse while weights stream in)
                for wg in range(4 if pi == 0 else 2):
                    fp_ = ps.tile([128, NT_F], F32, name="fillw",
                                  tag="mmps", bufs=4)
                    nc.tensor.matmul(fp_[:, :NT_F], wrm[:, 0:128],
                                     wrm[:, 128:632], start=True, stop=True)

        # kick AR1 first (its latency is the phase-2 critical path), THEN the
        # raw q/k/v weight reuse DMAs (transfers run during the AR; the
        # composite tiles' SBUF is dead after phase 1 and the framework
        # serializes the DMA behind phase 1's last reads), then the
        # queue-blocking readback.
        ar1_out = stats_kick_ar(st3, "1")
        for n, tg in (("wq", "wqc"), ("wk", "wkc"), ("wv", "wvc")):
            t_ = cpool.tile([128, 4 * F], FP16, name=n, tag=tg)
            nc.gpsimd.dma_start(t_[:], w_d[n][:])
            late_w[n] = t_
        arr1 = ar_read(ar1_out, "1")

        # fill the AR latency with AR-independent x-half convs: conv2x over
        # all chunks, interleaved with conv3x for the EARLY_SET chunks
        # (all-scalar conv3x evacs so nothing sits behind the vector queue
        # once stats_post blocks it).  The first two chunks reuse the x
        # tiles still resident from phase 1, so the window starts DMA-free.
        part2 = {}
        part3 = {}
        ld_engs = [nc.sync, nc.scalar]

        def filler(n_grp):
            # keep the PE streaming across the AR boundary so the HAM does
            # not re-throttle; results are discarded.  small groups so real
            # work enqueued later is not stuck behind a long fake stream.
            for fg in range(n_grp):
                fp_ = ps.tile([128, NT_F], F32, name="fill", tag="mmps",
                              bufs=4)
                for k in range(2):
                    nc.tensor.matmul(fp_[:, :NT_F],
                                     wsl(late_w['w2x'], k, fg % 4),
                                     late_w['w2x'][:, 0:NT_F],
                                     start=(k == 0), stop=(k == 1))

        for wi, ci in enumerate(WIN_ORDER):
            s0, ns = CHUNKS[ci]
            t0, nt = s0 * HW, ns * HW
            if ci in xn_keep:
                xn = xn_keep.pop(ci)
            else:
                xn = load_xn(xn_d, t0, nt, eng=ld_engs[wi % 2])
            if ci in WIN_ORDER[-2:]:
                xn_keep[ci] = xn   # still resident for the AR2 window
            for o in range(4):
                p = conv_gemm(late_w['w2x'], xn, nt, o)
                pt = work.tile([128, nt], FP16, name="part2",
                               tag=f"part{ci}_{o}", bufs=1)
                if o % 2 == 0:
                    nc.scalar.activation(pt[:, :nt], p[:, :nt], AF.Identity)
                else:
                    nc.vector.tensor_copy(pt[:, :nt], p[:, :nt])
                part2[(ci, o)] = pt
            if ci in EARLY_SET:
                for o in range(4):
                    p = conv_gemm(late_w['w3x'], xn, nt, o)
                    pt = work.tile([128, nt], FP16, name="part3e",
                                   tag=f"p3e{ci}_{o}", bufs=1)
                    nc.scalar.activation(pt[:, :nt], p[:, :nt], AF.Identity)
                    part3[(ci, o)] = pt
        filler(4)
        stats_post(arr1, [(sv["s2a"], sv["t2a"], 0, 4),
                          (sv["s3a"], sv["t3a"], 8, 12)])
        filler(3)

        # ---------------- phase 2 ----------------
        for ci, (s0, ns) in enumerate(CHUNKS):
            t0, nt = s0 * HW, ns * HW
            o3a = att_act(o3_att, t0, nt, sv["s2a"], sv["t2a"], "oa")
            o7 = []
            for o in range(4):
                p = conv_gemm(late_w['w2a'], o3a, nt, o)
                o7k = work.tile([128, NT_F], FP16, name="o7", tag=f"o3{o}")
                nc.vector.scalar_tensor_tensor(
                    o7k[:, :nt], p[:, :nt], vslice(o, "b2"),
                    part2[(ci, o)][:, :nt], ALU.add, ALU.add)
                o7.append(o7k)
            mhsa(o7, o7_att, t0, nt, ns, st7, ci, q_on_scalar=True)

        # kick AR2; fill with the remaining conv3 x+a work.  stats_post(arr2)
        # is emitted after the loop so its (blocked) vector ops cannot gate
        # the loop's own psum evacuations.
        ar2_out = stats_kick_ar(st7, "2")
        arr2 = ar_read(ar2_out, "2")
        for wi, ci in enumerate(AR2WIN_ORDER):
            s0, ns = CHUNKS[ci]
            t0, nt = s0 * HW, ns * HW
            o3a = att_act(o3_att, t0, nt, sv["s3a"], sv["t3a"], "oa")
            if ci in EARLY_SET:
                # a-half only; add in place into the stored x-half partial
                for o in range(4):
                    p = conv_gemm(late_w['w3a'], o3a, nt, o)
                    pt = part3[(ci, o)]
                    nc.vector.tensor_tensor(pt[:, :nt], p[:, :nt],
                                            pt[:, :nt], ALU.add)
            else:
                if ci in xn_keep:
                    xn = xn_keep.pop(ci)
                else:
                    xn = load_xn(xn_d, t0, nt, eng=ld_engs[wi % 2])
                for o in range(4):
                    p = ps.tile([128, NT_F], F32, name="mmps3", tag="mmps",
                                bufs=4)
                    for k in range(4):
                        nc.tensor.matmul(p[:, :nt], wsl(late_w['w3x'], k, o),
                                         xn[k][:, :nt], start=(k == 0),
                                         stop=False)
                    for k in range(4):
                        nc.tensor.matmul(p[:, :nt], wsl(late_w['w3a'], k, o),
                                         o3a[k][:, :nt], start=False,
                                         stop=(k == 3))
                    pt = work.tile([128, nt], FP16, name="part3",
                                   tag=f"part{ci}_{o}", bufs=1)
                    if o % 2 == 0 or wi >= len(AR2WIN_ORDER) - 2:
                        nc.scalar.activation(pt[:, :nt], p[:, :nt], AF.Identity)
                    else:
                        nc.vector.tensor_copy(pt[:, :nt], p[:, :nt])
                    part3[(ci, o)] = pt
        stats_post(arr2, [(sv["s3b"], sv["t3b"], 16, 20)])
        filler(2)

        # ---------------- phase 3 ----------------
        # natural order: the kernel ends on the small remainder chunk, whose
        # output DMA drains in ~1us (full chunks spread over three queues).
        out_engs = [nc.gpsimd, nc.sync, nc.scalar]
        for pi, ci in enumerate(range(NCH)):
            s0, ns = CHUNKS[ci]
            t0, nt = s0 * HW, ns * HW
            o7a = att_act(o7_att, t0, nt, sv["s3b"], sv["t3b"], "oa")
            for o in range(4):
                p = conv_gemm(late_w['w3b'], o7a, nt, o)
                ot = work.tile([128, NT_F], FP16, name="ot", tag=f"ot{o}",
                               bufs=1)
                nc.vector.scalar_tensor_tensor(
                    ot[:, :nt], p[:, :nt], vslice(o, "b3"),
                    part3[(ci, o)][:, :nt], ALU.add, ALU.add)
                out_engs[(pi * 4 + o) % 3].dma_start(out_d[:, o, t0:t0 + nt],
                                                     ot[:, :nt])
        es.close()

    nc.compile()
    return nc


def _host_prep(inputs):
    g = {k: np.asarray(v, np.float32) for k, v in inputs.items()}
    x = g["x"]
    m = x.mean(axis=(0, 2, 3))
    v = x.var(axis=(0, 2, 3))
    rs = 1.0 / np.sqrt(v + EPS)

    def st(gam, bet):
        s = gam * rs
        return s, bet - m * s

    hf = np.float16

    def xn_prep(gam, bet):
        s, t = st(gam, bet)
        xa = np.maximum(x * s[None, :, None, None] + t[None, :, None, None],
                        0.0).astype(hf)
        # [B, C, 3, 3] -> [C, B, HW] -> per-core [128, 4, T]
        return np.ascontiguousarray(xa.reshape(B, C, HW).transpose(1, 0, 2))

    # the kernel shares one pre-activated x across all three phases; valid
    # because the three BNs carry identical gamma/beta on the x channels.
    assert np.array_equal(g["bn2_g"][:C], g["bn1_g"])
    assert np.array_equal(g["bn2_b"][:C], g["bn1_b"])
    assert np.array_equal(g["bn3_g"][:C], g["bn1_g"])
    assert np.array_equal(g["bn3_b"][:C], g["bn1_b"])
    xn1 = xn_prep(g["bn1_g"], g["bn1_b"])

    # composite MHSA1 projection weights: q = (Wq W1) a1 + (Wq b1 + qb)
    wq_c = g["q_w"] @ g["w1"]
    wk_c = g["k_w"] @ g["w1"]
    wv_c = g["v_w"] @ g["w1"]
    qb_c = g["q_w"] @ g["b1"] + g["q_b"]
    kb_c = g["k_w"] @ g["b1"] + g["k_b"]
    # v bias (and the folded W1 bias through v) cancels in BN downstream.

    vec_cols = {}
    vec_cols["qb"] = qb_c
    vec_cols["kb"] = kb_c
    vec_cols["b2"] = g["b2"]
    vec_cols["b3"] = g["b3"]
    vec_cols["g2a"] = g["bn2_g"][C:]
    vec_cols["b2a"] = g["bn2_b"][C:]
    vec_cols["g3a"] = g["bn3_g"][C:2 * C]
    vec_cols["b3a"] = g["bn3_b"][C:2 * C]
    vec_cols["g3b"] = g["bn3_g"][2 * C:]
    vec_cols["b3b"] = g["bn3_b"][2 * C:]
    vec_cols["c6"] = np.full(C, -9.0, np.float32)
    vecs = np.zeros((128, 4, NV), np.float32)
    for n, i in VEC.items():
        vecs[:, :, i] = vec_cols[n].reshape(4, 128).T
    vec4 = np.zeros((128, 24), np.float32)
    for j, n in enumerate(["g2a", "b2a", "g3a", "b3a", "g3b", "b3b"]):
        vec4[:, 4 * j:4 * j + 4] = vec_cols[n].reshape(4, 128).T

    def wtile(wT):
        # [C_in_512, F] -> [128, 4*F]
        return np.ascontiguousarray(
            wT.reshape(4, 128, F).transpose(1, 0, 2).reshape(128, 4 * F)
        ).astype(hf)

    pos = (g["rel_h"] + g["rel_w"]).reshape(HEADS, D, HW)
    posrep = np.tile(pos, (1, 1, 14)).transpose(1, 0, 2).reshape(D, HEADS * GS)

    b_of = np.repeat(np.arange(14), HW)
    blk1 = (b_of[:, None] == b_of[None, :]).astype(np.float32)

    shared = {
        "wqc": wtile(wq_c.T),
        "wkc": wtile(wk_c.T),
        "wvc": wtile(wv_c.T),
        "wq": wtile(g["q_w"].T),
        "wk": wtile(g["k_w"].T),
        "wv": wtile(g["v_w"].T),
        "w2x": wtile(g["w2"].T[:C]),
        "w2a": wtile(g["w2"].T[C:]),
        "w3x": wtile(g["w3"].T[:C]),
        "w3a": wtile(g["w3"].T[C:2 * C]),
        "w3b": wtile(g["w3"].T[2 * C:]),
        "vecs": vecs.reshape(128, 4 * NV),
        "vec4": vec4,
        "posrep": posrep.astype(hf),
        "blkmask": np.tile(blk1, (1, 4)).astype(hf),
        "ones126": np.ones((GS, 128), np.float32).astype(hf),
    }
    in_maps = []
    for c in range(N_CORES):
        def core_x(xa):
            xs = xa[:, BC * c:BC * (c + 1), :].reshape(C, T)
            return np.ascontiguousarray(
                xs.reshape(4, 128, T).transpose(1, 0, 2))
        in_maps.append(dict(shared, xn=core_x(xn1)))
    return in_maps


def kernel(**inputs):
    if "nc" not in _cache:
        _cache["nc"] = _build()
    nc = _cache["nc"]
    in_maps = _host_prep(inputs)
    res = run_bass_kernel_spmd(nc, in_maps, core_ids=list(range(N_CORES)))
    parts = [res.results[c]["out_cm"].astype(np.float32)
             .reshape(128, 4, BC, HW).transpose(1, 0, 2, 3).reshape(F, BC, HW)
             for c in range(N_CORES)]
    full = np.concatenate(parts, axis=1)          # [F, B, HW]
    return np.ascontiguousarray(full.transpose(1, 0, 2)).reshape(B, F, 3, 3)
